# revision 7
# baseline (speedup 1.0000x reference)
"""Trainium2 Bass kernel for nn_ContinuousPool.

Computes, for x:(32,96,128,128) f32 and pool_strength:(1,96,1,1) f32:
    cur = x
    repeat 10: cur = cur + s * (maxpool3x3_same(cur) - cur)
    out = avgpool2x2(cur)            -> (32,96,64,64)

Strategy (v3):
  - Pure data parallel over 8 cores: 4 batches/core -> 384 images/core,
    processed as 3 chunks of 128 images (one image per SBUF partition).
  - State in fp16: DVE tensor_tensor runs in 2x_1p mode (2 elem/cycle,
    measured 8.4us per 16384-elem pass vs 17.7us for f32). End-to-end
    fp16 error is ~1.2e-3 vs the 2e-2 tolerance.
  - The rescaled blend u' = u + c*max3x3(u), c = s/(1-s), makes each step
    4 tensor_max + 1 scalar_tensor_tensor (measured 11.2us); the
    (1-s)^10 / 4 factor folds into the final avgpool scale.
  - All compute on DVE (Pool-engine tensor ops don't compile through
    neuronxcc; Act-engine fp16 output is pathologically slow). Input
    conversion f32->fp16 is a DVE tensor_copy from a staged half-chunk,
    preloaded by DMA during the previous chunk's steps.

Layouts per partition (one 128x128 image each):
  u: [128 rows x 132] fp16, interior at cols 2..129, NEG pads at cols 1,130
  r: [130 rows x 128] fp16 (row maxes), NEG pad rows 0 and 129
  v: [128 x 128] fp16 (column maxes / scaled update)
"""

import sys

import numpy as np

if "/opt/trn_rl_repo" not in sys.path:
    sys.path.insert(0, "/opt/trn_rl_repo")

B, C, H, W = 32, 96, 128, 128
T = 10
N_CORES = 8
B_PER_CORE = B // N_CORES          # 4
IMGS = B_PER_CORE * C              # 384 images per core
CHUNK = 128                        # images (partitions) per chunk
NCHUNK = IMGS // CHUNK             # 3
SU = 132                           # u row stride (elements)
SR = 128                           # r row stride
NEG = -60000.0                     # -inf stand-in that fits fp16

_CACHE = {}


def _build_program(reps=None):
    import concourse.bacc as bacc
    import concourse.mybir as mybir
    from concourse import tile

    f16 = mybir.dt.float16
    f32 = mybir.dt.float32
    alu = mybir.AluOpType

    nc = bacc.Bacc("TRN2", target_bir_lowering=False, debug=False,
                   num_devices=N_CORES)

    x_d = nc.dram_tensor("x", [IMGS, H * W], f32, kind="ExternalInput")
    c_d = nc.dram_tensor("cvec", [IMGS, 1], f32, kind="ExternalInput")
    f_d = nc.dram_tensor("fvec", [IMGS, 1], f32, kind="ExternalInput")
    y_d = nc.dram_tensor("y", [IMGS, (H // 2) * (W // 2)], f32,
                         kind="ExternalOutput")

    with tile.TileContext(nc, num_cores=N_CORES) as tc:
        with tc.tile_pool(name="main", bufs=1) as pool:
            u_ts = [pool.tile([128, H * SU], f16, name=f"u{i}", tag=f"u{i}")
                    for i in (0, 1)]
            r_t = pool.tile([128, 130 * SR], f16, tag="r")
            v_t = pool.tile([128, H * W], f16, tag="v")
            st_ts = [pool.tile([128, (H // 2) * W], f32, name=f"st{i}",
                               tag=f"st{i}") for i in (0, 1)]
            cs_t = pool.tile([128, 2 * NCHUNK], f32, tag="cs")

            # one-time pad init; interiors are rewritten per chunk
            for u_t in u_ts:
                nc.gpsimd.memset(u_t[:, :], NEG)
            nc.gpsimd.memset(r_t[:, :], NEG)
            for k in range(NCHUNK):
                rows = slice(k * CHUNK, (k + 1) * CHUNK)
                nc.sync.dma_start(cs_t[:, 2 * k:2 * k + 1], c_d[rows, :])
                nc.sync.dma_start(cs_t[:, 2 * k + 1:2 * k + 2], f_d[rows, :])

            def uv(u_t, h0, h1, c0, c1):
                """[128, h1-h0, c1-c0] view of u rows h0..h1, cols c0..c1."""
                t = u_t[:, h0 * SU:h1 * SU]
                return t.rearrange("p (h w) -> p h w", h=h1 - h0,
                                   w=SU)[:, :, c0:c1]

            def rv(h0, h1):
                """[128, h1-h0, 128] view of r rows h0..h1 (130 rows)."""
                t = r_t[:, h0 * SR:h1 * SR]
                return t.rearrange("p (h w) -> p h w", h=h1 - h0, w=SR)

            def vv():
                return v_t[:, :].rearrange("p (h w) -> p h w", h=H, w=W)

            def st_view(half):
                hh = H // 2
                return st_ts[half][:, :].rearrange("p (h w) -> p h w",
                                                   h=hh, w=W), half * hh

            def dma_in(k, half):
                """DMA half of chunk k's x (f32) into its stage buffer."""
                rows = slice(k * CHUNK, (k + 1) * CHUNK)
                x_v = x_d[rows, :].rearrange("p (h w) -> p h w", h=H, w=W)
                st_v, h0 = st_view(half)
                nc.sync.dma_start(st_v, x_v[:, h0:h0 + H // 2, :])

            def convert(u_t, half):
                """fp16-convert the staged half into u_t's interior."""
                st_v, h0 = st_view(half)
                nc.vector.tensor_copy(uv(u_t, h0, h0 + H // 2, 2, 130), st_v)

            def step(u_t, k):
                # row max3 into r rows 1..129 (2 passes, 2x each)
                nc.vector.tensor_max(rv(1, 129), uv(u_t, 0, H, 1, 129),
                                     uv(u_t, 0, H, 3, 131))
                nc.vector.tensor_max(rv(1, 129), rv(1, 129),
                                     uv(u_t, 0, H, 2, 130))
                # col max3 into v (2 passes)
                nc.vector.tensor_max(vv(), rv(0, 128), rv(2, 130))
                nc.vector.tensor_max(vv(), vv(), rv(1, 129))
                # u += c * v in one scalar_tensor_tensor
                nc.vector.scalar_tensor_tensor(
                    uv(u_t, 0, H, 2, 130), vv(), cs_t[:, 2 * k:2 * k + 1],
                    uv(u_t, 0, H, 2, 130), op0=alu.mult, op1=alu.add)

            def epilogue(u_t, k):
                # avgpool 2x2 * f  ->  o_t (f32), then DMA out
                u4 = u_t[:, 0:H * SU].rearrange(
                    "p (h w2 two) -> p h w2 two", h=H, w2=SU // 2, two=2)
                v3 = vv()
                # horizontal pairs -> v[:, :, 0:64]
                nc.vector.tensor_add(v3[:, :, 0:64], u4[:, :, 1:65, 0:1],
                                     u4[:, :, 1:65, 1:2])
                # vertical pairs -> v[:, 0:64, 64:128]
                a2 = v_t[:, 0:H * W].rearrange(
                    "p (h2 two w) -> p h2 two w", h2=H // 2, two=2, w=W)
                hv = H // 2
                nc.vector.tensor_add(v3[:, 0:hv, 64:128],
                                     a2[:, :, 0:1, 0:64],
                                     a2[:, :, 1:2, 0:64])
                # scale by (1-s)^10/4, f32 output into the (now dead) r
                # tile via an f32 view, offset past r's pad row 0
                r32 = r_t.bitcast(mybir.dt.float32)
                o_v = r32[:, SR // 2:SR // 2 + hv * 64].rearrange(
                    "p (h w) -> p h w", h=hv, w=64)
                nc.vector.tensor_scalar_mul(o_v, v3[:, 0:hv, 64:128],
                                            cs_t[:, 2 * k + 1:2 * k + 2])
                rows = slice(k * CHUNK, (k + 1) * CHUNK)
                nc.sync.dma_start(
                    y_d[rows, :].rearrange("p (h w) -> p h w", h=hv, w=64),
                    o_v)

            def body():
                dma_in(0, 0)
                dma_in(0, 1)
                for k in range(NCHUNK):
                    u_t = u_ts[k % 2]
                    # convert this chunk's staged halves, then free the
                    # stage for the next chunk's DMA (WAR-tracked)
                    convert(u_t, 0)
                    convert(u_t, 1)
                    if k + 1 < NCHUNK:
                        dma_in(k + 1, 0)
                        dma_in(k + 1, 1)
                    for _ in range(T):
                        step(u_t, k)
                    epilogue(u_t, k)

            if reps is None:
                body()
            else:
                with tc.For_i(0, reps):
                    body()

    nc.compile()
    return nc


def build_program(reps=None):
    key = ("nc", reps)
    if key not in _CACHE:
        _CACHE[key] = _build_program(reps)
    return _CACHE[key]


def kernel(x: np.ndarray, pool_strength: np.ndarray) -> np.ndarray:
    from concourse.bass_utils import run_bass_kernel_spmd

    nc = build_program()

    x = np.asarray(x, dtype=np.float32)
    s = np.asarray(pool_strength, dtype=np.float64).reshape(C)
    c_ch = (s / (1.0 - s)).astype(np.float32)                  # [C]
    f_ch = (((1.0 - s) ** T) * 0.25).astype(np.float32)        # [C]
    cvec = np.ascontiguousarray(np.tile(c_ch, B_PER_CORE)[:, None])  # [384,1]
    fvec = np.ascontiguousarray(np.tile(f_ch, B_PER_CORE)[:, None])

    in_maps = []
    for j in range(N_CORES):
        xj = np.ascontiguousarray(
            x[j * B_PER_CORE:(j + 1) * B_PER_CORE].reshape(IMGS, H * W))
        in_maps.append({"x": xj, "cvec": cvec, "fvec": fvec})

    res = run_bass_kernel_spmd(nc, in_maps, list(range(N_CORES)))

    out = np.empty((B, C, H // 2, W // 2), dtype=np.float32)
    for j in range(N_CORES):
        yj = res.results[j]["y"].reshape(B_PER_CORE, C, H // 2, W // 2)
        out[j * B_PER_CORE:(j + 1) * B_PER_CORE] = yj
    return out
